# revision 74
# baseline (speedup 1.0000x reference)
"""Cross-attention adapter kernel for Trainium2 (8 NeuronCores).

Sharding: data-parallel over batch (B=2) x tensor-parallel over heads
(4 groups of 4 heads). Core c handles batch c//4, head group c%4. Each
core computes rmsnorm + q/k/v projections for its head slice, attention,
and a partial o-projection; the host sums the 4 partials per batch.

v3 scheduling (vs v2 baseline at ~334us; measured ~262us):
- k/v projections run on RAW kv (no in-place normalize): rsk is folded
  into the kT psum->sbuf copies (tensor_mul with an rsk broadcast) and
  into the v copies (per-partition tensor_scalar with a transposed rsk
  column built from 5 ap=1 PE transposes; Wv is host-prescaled by
  sqrt(HD) so one column serves both). Removes the kv-normalize DVE
  serialization and its PE stall, and lets k-proj start as soon as
  wk/kv DMAs land.
- rsk itself is exp(-0.5*ln(ssq/D+eps)) on ACT (Ln/Exp share the Exp
  table -> no ACT table swaps anywhere in the kernel).
- Per-chunk emission interleaves q-proj m-tiles between per-head
  scores groups so the PE never waits on the ACT exp chain; ssq
  matmuls are emitted late (their squares long done on vector).
- kv is zero-padded to 640 tokens host-side so scores / attn_v /
  v-proj tail tiles run full-width (128 partitions) instead of the
  slow 64-row path; the mask bias covers the pad.
- h squares for prefetched chunks run on vector; h0/h1 squares in the
  kv phase run on the then-idle ACT engine. hp pool holds 3 chunk
  buffers so the h(qc+2) DMA is never blocked by q-proj(qc+1).
- o-proj partials are written as bf16 (host sums in fp32): halves the
  output DMA; the last chunk's final out-DMAs split across two queues.
- Denominator rows for 2 heads share one PSUM bank (partitions 0/64);
  the hssq row shares the sums0 bank (temporally disjoint), freeing a
  bank for a 4-deep scores/attn_v/o-proj psum rotation.

Known plateau: PE busy ~229us (bf16 matmul-column floor ~206us) +
~12us startup DMA + ~13us residual gaps + ~7us drain tail. fp8 was
evaluated and rejected: e4m3 quantization noise exceeds the 2e-2
correctness gate (measured 5.8-11% in simulation).
"""

import sys

if "/opt/trn_rl_repo" not in sys.path:
    sys.path.insert(0, "/opt/trn_rl_repo")

import os
import types
from contextlib import ExitStack

import numpy as np


def _install_ntff_shim():
    """Make `antenv.axon_hooks` importable when the image lacks it, so
    run_bass_kernel_spmd(trace=True) can capture NTFF profiles under
    axon. No-op when the real module exists or axon boot is absent."""
    try:
        import antenv.axon_hooks  # noqa: F401
        return
    except ImportError:
        pass
    try:
        import antenv
        from trn_agent_boot.trn_boot import _ntff_profile_via_ctypes
        hook = _ntff_profile_via_ctypes("/opt/axon/libaxon_pjrt.so")
        mod = types.ModuleType("antenv.axon_hooks")
        mod._hook = hook
        mod.get_axon_ntff_profile_hook = lambda: mod._hook
        mod.set_axon_ntff_profile_hook = lambda h: setattr(mod, "_hook", h)
        import sys as _sys
        _sys.modules["antenv.axon_hooks"] = mod
        antenv.axon_hooks = mod
    except Exception:
        pass


_install_ntff_shim()

import concourse.mybir as mybir
import concourse.tile as tile
from concourse import bacc
from concourse.bass_utils import run_bass_kernel_spmd

# Problem shapes (hardcoded per contest contract).
B = 2
T = 2048
D = 2048
H = 16
HD = 128
N_WS = 64
T_CORR = 512
SKV = N_WS + T_CORR  # 576
EPS = 1e-6

# Sharding config.
G = 4                  # tensor-parallel head groups
HL = H // G            # 4 local heads per core
INNER_L = HL * HD      # 512
IT = INNER_L // 128    # 4  m-tiles of local inner dim
TQ = 512               # query-token chunk
NCH = T // TQ          # 4
DT = D // 128          # 16 contraction tiles over D
DP = DT // 2           # 8 dt-pair tiles
KVT = (SKV + 127) // 128       # 5 kv partition tiles
KV_PAD = KVT * 128             # 640
DN = D // 512          # 4  output N-tiles
SCALE = 1.0 / float(np.sqrt(HD))
SQRT_HD = float(np.sqrt(HD))
NEG = -1.0e9

F32 = mybir.dt.float32
BF16 = mybir.dt.bfloat16
EXP = mybir.ActivationFunctionType.Exp
SQUARE = mybir.ActivationFunctionType.Square
MUL = mybir.AluOpType.mult
ADD = mybir.AluOpType.add

_NC = None
LAST_RESULTS = None


def _kv_p(mt):
    return min(128, SKV - mt * 128)


class _Em:
    def __init__(self, nc, tc, tensors, ctx):
        self.nc = nc
        self.tc = tc
        (self.hT, self.kvT, self.wq, self.wk, self.wv, self.wo,
         self.maskb, self.out) = tensors

        p = lambda name, bufs: ctx.enter_context(
            tc.tile_pool(name=name, bufs=bufs))
        self.const = p("const", 1)
        self.rows = p("rows", 4)
        self.bcast = p("bc", 2)       # per-tag bufs
        self.ktp = p("kt", 1)
        self.vp = p("v", 1)
        self.wqp = p("wq", 1)
        self.wop = p("wo", 1)
        self.hp = p("hp", 3)
        self.hsqp = p("hsq", 1)       # per-dt tags, h squares all phases
        self.qp = p("qp", 1)
        self.otp = p("otp", 1)
        self.fop = p("fop", 4)
        self.scp = p("scp", 1)
        # transient pools (kv phase / chunk scope), set later
        self.sqp = None
        self.attnp = None

        self.h_ts = {}
        self.rsq = {}
        self.qT = {}
        self.kT_sb = None
        self.v_sb = None
        self.scale_col = None
        self.attn = {}
        self.rsum_b = {}
        self.fo_eng = 0

        # constants
        nc_ = self.nc
        self.ones_f = self.const.tile([128, 1], F32)
        nc_.vector.memset(self.ones_f[:], 1.0)
        self.ones_t = self.const.tile([128, 1], BF16)
        nc_.vector.tensor_copy(self.ones_t[:], self.ones_f[:])
        self.eps_row = self.const.tile([1, 1], F32)
        nc_.vector.memset(self.eps_row[:], EPS)
        self.zero_row = self.const.tile([1, 1], F32)
        nc_.vector.memset(self.zero_row[:], 0.0)
        self.maskb_sb = self.const.tile([128, KVT], F32)
        nc_.scalar.dma_start(
            out=self.maskb_sb[:],
            in_=self.maskb.rearrange("(mt p) one -> p (mt one)", p=128),
        )


    # -- h pipeline ----------------------------------------------------
    def h_dma(self, qc):
        nc = self.nc
        ht = self.hp.tile([128, DP, 2, TQ], BF16, tag="h", name=f"h{qc}")
        for dp in range(DP):
            nc.sync.dma_start(out=ht[:, dp, :, :], in_=self.hT[qc, dp])
        self.h_ts[qc] = ht
        return ht

    def h_squares(self, qc, engines):
        """Square h dt-tiles; engines is a list cycled over dt."""
        nc = self.nc
        ht = self.h_ts[qc]
        sqs = []
        for dt in range(DT):
            sq = self.hsqp.tile([128, TQ], BF16, tag=f"hsq{dt}",
                                name=f"hsq{qc}_{dt}")
            eng = engines[dt % len(engines)]
            if eng is nc.scalar:
                nc.scalar.activation(sq[:], ht[:, dt // 2, dt % 2, :],
                                     SQUARE)
            else:
                eng.tensor_mul(sq[:], ht[:, dt // 2, dt % 2, :],
                               ht[:, dt // 2, dt % 2, :])
            sqs.append(sq)
        return sqs

    def h_ssq_mm(self, qc, sqs):
        nc = self.nc
        ssq = self.ps_row.tile([128, TQ], F32, tag=self.row_tag,
                               name=f"ssq{qc}")
        for dt in range(DT):
            nc.tensor.matmul(
                ssq[0:1, :], self.ones_t[:], sqs[dt][:],
                start=(dt == 0), stop=(dt == DT - 1),
            )
        return ssq

    def h_rsq(self, qc, ssq):
        """rsq = 1/sqrt(ssq/D + eps) via two Newton steps from y0=1."""
        nc = self.nc
        y1 = self.rows.tile([1, SKV], F32, tag="srow", name=f"y1h{qc}")
        m = self.rows.tile([1, SKV], F32, tag="srow", name=f"mh{qc}")
        t = self.rows.tile([1, SKV], F32, tag="srow", name=f"th{qc}")
        nc.vector.tensor_scalar(y1[0:1, 0:TQ], ssq[0:1, :],
                                -0.5 / D, 1.5 - 0.5 * EPS, MUL, ADD)
        nc.vector.tensor_scalar(m[0:1, 0:TQ], ssq[0:1, :],
                                1.0 / D, EPS, MUL, ADD)
        nc.vector.tensor_mul(t[0:1, 0:TQ], y1[0:1, 0:TQ], y1[0:1, 0:TQ])
        nc.vector.tensor_mul(t[0:1, 0:TQ], t[0:1, 0:TQ], m[0:1, 0:TQ])
        nc.vector.tensor_scalar(t[0:1, 0:TQ], t[0:1, 0:TQ],
                                -0.5, 1.5, MUL, ADD)
        nc.vector.tensor_mul(y1[0:1, 0:TQ], y1[0:1, 0:TQ], t[0:1, 0:TQ])
        rsq_b = self.bcast.tile([128, SKV], F32, tag="rsqb",
                                name=f"rsqb{qc}")
        nc.gpsimd.partition_broadcast(rsq_b[:, 0:TQ], y1[0:1, 0:TQ])
        self.rsq[qc] = rsq_b

    # -- projections ---------------------------------------------------
    def q_proj_m(self, qc, m):
        nc = self.nc
        ht = self.h_ts[qc]
        pq = self.ps_q.tile([128, 512], F32, tag="qmm", bufs=self.q_bufs)
        for dt in range(DT):
            nc.tensor.matmul(
                pq[:, 0:TQ],
                self.wq_sb[:, m, dt, :],
                ht[:, dt // 2, dt % 2, :],
                start=(dt == 0), stop=(dt == DT - 1),
            )
        qt = self.qp.tile([128, TQ], BF16, tag=f"q{m}", name=f"q{qc}_{m}")
        nc.vector.tensor_mul(qt[:], pq[:, 0:TQ], self.rsq[qc][:, 0:TQ])
        self.qT.setdefault(qc, [None] * IT)[m] = qt
        if m == IT - 1:
            self.rsq.pop(qc)

    # -- attention -----------------------------------------------------
    def scores(self, qc, hl):
        """scores + exp for one head; exp folds SCALE*rsk per kv row."""
        nc = self.nc
        qt = self.qT[qc][hl]
        attn_sb = self.attnp.tile([128, KVT, TQ], BF16, tag=f"attn{hl}",
                                  name=f"attn{qc}_{hl}")
        for mt in range(KVT):
            ps = self.ps_mm.tile([128, 512], F32, tag="mm")
            nc.tensor.matmul(
                ps[:, 0:TQ],
                self.kT_sb[hl][:, mt * 128: mt * 128 + 128],
                qt[:],
                start=True, stop=True,
            )
            nc.scalar.activation(
                attn_sb[:, mt, :], ps[:, 0:TQ], EXP,
                bias=self.maskb_sb[:, mt: mt + 1], scale=SCALE,
            )
        self.attn[(qc, hl)] = attn_sb

    def denoms(self, qc, hl):
        """ones-matmul row sums for one head + reciprocal broadcast.
        Two heads share one PSUM bank (rows 0 / 64)."""
        nc = self.nc
        attn_sb = self.attn[(qc, hl)]
        grp = hl // 2
        if hl % 2 == 0:
            self.sums[grp] = self.ps_sums.tile(
                [128, TQ], F32, tag=f"sums{grp}", bufs=1,
                name=f"sums{qc}_{grp}")
        srow = self.sums[grp][(hl % 2) * 64: (hl % 2) * 64 + 1, :]
        for mt in range(KVT):
            nc.tensor.matmul(
                srow, self.ones_t[:], attn_sb[:, mt, :],
                start=(mt == 0), stop=(mt == KVT - 1),
            )
        rsb = self.rows.tile([1, SKV], F32, tag="srow",
                             name=f"rsb{qc}_{hl}")
        rs = self.rows.tile([1, SKV], F32, tag="srow",
                            name=f"rs{qc}_{hl}")
        nc.vector.tensor_copy(rsb[0:1, 0:TQ], srow)
        nc.vector.reciprocal_approx_fast(rs[0:1, 0:TQ], rsb[0:1, 0:TQ])
        rb = self.bcast.tile([128, SKV], F32, tag=f"rsum{hl % 2}",
                             name=f"rsb{qc}_{hl}")
        nc.gpsimd.partition_broadcast(rb[:, 0:TQ], rs[0:1, 0:TQ])
        self.rsum_b[(qc, hl)] = rb

    def attn_v_h(self, qc, hl):
        nc = self.nc
        attn_sb = self.attn.pop((qc, hl))
        po = self.ps_mm.tile([128, 512], F32, tag="mm")
        for mt in range(KVT):
            nc.tensor.matmul(
                po[:, 0:TQ],
                self.v_sb[mt][:, hl * 128: (hl + 1) * 128],
                attn_sb[:, mt, :],
                start=(mt == 0), stop=(mt == KVT - 1),
            )
        ot = self.otp.tile([128, TQ], BF16, tag=f"ot{hl}",
                           name=f"ot{qc}_{hl}")
        rb = self.rsum_b.pop((qc, hl))
        nc.vector.tensor_mul(ot[:], po[:, 0:TQ], rb[:, 0:TQ])
        return ot

    # -- o-projection --------------------------------------------------
    def o_proj_units(self, qc, outT_sb, units):
        """Emit a subset of the 16 (tt, n) o-projection units."""
        nc = self.nc
        for u in units:
            tt, n = u // DN, u % DN
            pf = self.ps_mm.tile([128, 512], F32, tag="mm")
            for m in range(IT):
                nc.tensor.matmul(
                    pf[:],
                    outT_sb[m][:, tt * 128: (tt + 1) * 128],
                    self.wo_sb[:, n, m, :],
                    start=(m == 0), stop=(m == IT - 1),
                )
            fo = self.fop.tile([128, 512], BF16, tag="fo")
            e = self.fo_eng % 2
            self.fo_eng += 1
            if e == 0:
                nc.scalar.copy(fo[:], pf[:])
                deng = nc.scalar
            else:
                nc.vector.tensor_copy(fo[:], pf[:])
                deng = nc.gpsimd
            t0 = qc * TQ + tt * 128
            if qc == NCH - 1 and u >= 8:
                # tail: split across two queues to halve drain time
                deng.dma_start(
                    out=self.out[t0: t0 + 64, n * 512: (n + 1) * 512],
                    in_=fo[0:64, :],
                )
                deng2 = nc.gpsimd if deng is nc.scalar else nc.scalar
                deng2.dma_start(
                    out=self.out[t0 + 64: t0 + 128,
                                 n * 512: (n + 1) * 512],
                    in_=fo[64:128, :],
                )
            else:
                deng.dma_start(
                    out=self.out[t0: t0 + 128, n * 512: (n + 1) * 512],
                    in_=fo[:],
                )

    # -- kv phase ------------------------------------------------------
    def kv_phase(self):
        nc, tc = self.nc, self.tc
        with tc.tile_pool(name="kvps", bufs=5, space="PSUM") as kv_mm, \
             tc.tile_pool(name="kvrow", bufs=1, space="PSUM") as ps_row, \
             tc.tile_pool(name="rskt", bufs=1, space="PSUM") as ps_rskt, \
             nc.named_scope("kvphase"):
            self.ps_mm = kv_mm
            self.ps_q = kv_mm
            self.q_bufs = 1
            self.ps_row = ps_row
            self.row_tag = "row"
            self._kv_phase_body(ps_rskt)

    def _kv_phase_body(self, ps_rskt):
        nc, tc = self.nc, self.tc
        self.kT_sb = [
            self.ktp.tile([128, KV_PAD], BF16, tag=f"kt{m}", name=f"kt{m}")
            for m in range(IT)
        ]
        self.v_sb = [
            self.vp.tile([128, INNER_L], BF16, tag=f"v{mt}", name=f"v{mt}")
            for mt in range(KVT)
        ]
        # zero the kv padding (tokens 576..639) so the padded scores /
        # attn_v tiles contribute exactly zero
        for m in range(IT):
            nc.gpsimd.memset(self.kT_sb[m][:, SKV:KV_PAD], 0.0)
        nc.gpsimd.memset(self.v_sb[KVT - 1][64:128, :], 0.0)
        wk_r = self.wk.rearrange("m p dt c -> p m dt c")
        wq_r = self.wq.rearrange("m p dt c -> p m dt c")
        wo_r = self.wo.rearrange("n p m c -> p n m c")
        with tc.tile_pool(name="kvp", bufs=1) as kvp, \
             tc.tile_pool(name="sqkv", bufs=1) as sqkv:
            kv_t = kvp.tile([128, DP, 2, KV_PAD], BF16, tag="kv", name="kv")

            def kvn(dt):
                return kv_t[:, dt // 2, dt % 2, :]

            with tc.tile_pool(name="wkp", bufs=1) as wkp:
                # DMA order: wk(m)/kv(dp) interleaved so k-proj's first
                # m-pass is fed earliest
                wk_sb = wkp.tile([128, IT, DT, 128], BF16, tag="wk")
                for m in range(IT):
                    for hh in range(2):
                        nc.sync.dma_start(
                            out=wk_sb[:, m, hh * 8:(hh + 1) * 8, :],
                            in_=wk_r[:, m, hh * 8:(hh + 1) * 8, :])
                    for dp in (2 * m, 2 * m + 1):
                        nc.sync.dma_start(out=kv_t[:, dp, :, :],
                                          in_=self.kvT[dp])
                self.h_dma(0)
                self.wq_sb = self.wqp.tile([128, IT, DT, 128], BF16,
                                           tag="wq")
                for m in range(IT):
                    nc.sync.dma_start(out=self.wq_sb[:, m, :, :],
                                      in_=wq_r[:, m, :, :])
                self.h_dma(1)
                self.wo_sb = self.wop.tile([128, DN, IT, 512], BF16,
                                           tag="wo")
                for n in range(DN):
                    nc.sync.dma_start(out=self.wo_sb[:, n, :, :],
                                      in_=wo_r[:, n, :, :])

                self.rsk_b = self.bcast.tile([128, SKV], F32, tag="rskb",
                                             bufs=1, name="rskb")

                def kproj_mms(ms):
                    pks = {}
                    for m in ms:
                        for i in range(2):
                            pks[(m, i)] = self.ps_mm.tile(
                                [128, 512], F32, tag="mm",
                                name=f"pk{m}_{i}")
                    for dt in range(DT):
                        for m in ms:
                            for i, (s0, s1) in enumerate(
                                    ((0, 288), (288, SKV))):
                                nc.tensor.matmul(
                                    pks[(m, i)][:, 0: s1 - s0],
                                    wk_sb[:, m, dt, :],
                                    kvn(dt)[:, s0:s1],
                                    start=(dt == 0), stop=(dt == DT - 1),
                                )
                    return pks

                def kproj_copies(ms, pks):
                    # rsk folded at copy time via rsk_b broadcast
                    for m in ms:
                        for i, (s0, s1) in enumerate(((0, 288), (288, SKV))):
                            nc.vector.tensor_mul(
                                self.kT_sb[m][:, s0:s1],
                                pks[(m, i)][:, 0: s1 - s0],
                                self.rsk_b[:, s0:s1])

                pks0 = kproj_mms((0, 1))

                # kv squares on vector/gpsimd; ssq matmuls accumulate two
                # 288-halves at psum partitions 0/64
                ssk = self.ps_row.tile([128, 512], F32, tag="row",
                                       name="ssk")
                for dt in range(DT):
                    sq = sqkv.tile([128, SKV], BF16, tag=f"kvsq{dt}",
                                   name=f"kvsq{dt}")
                    eng = nc.vector if dt % 2 == 0 else nc.gpsimd
                    eng.tensor_mul(sq[:], kvn(dt)[:, 0:SKV],
                                   kvn(dt)[:, 0:SKV])
                    nc.tensor.matmul(
                        ssk[0:1, 0:288], self.ones_t[:], sq[:, 0:288],
                        start=(dt == 0), stop=(dt == DT - 1),
                    )
                    nc.tensor.matmul(
                        ssk[64:65, 0:288], self.ones_t[:], sq[:, 288:SKV],
                        start=(dt == 0), stop=(dt == DT - 1),
                    )

                # rsk = exp(-0.5*ln(ssq/D + eps)) on ACT (Ln/Exp share
                # the Exp table -> no table swap; much shorter chain
                # than DVE newton rows)
                lnk = self.rows.tile([1, SKV], F32, tag="srow",
                                     name="lnk")
                LN = mybir.ActivationFunctionType.Ln
                nc.scalar.activation(lnk[0:1, 0:288],
                                     ssk[0:1, 0:288], LN,
                                     bias=self.eps_row[:], scale=1.0 / D)
                nc.scalar.activation(lnk[0:1, 288:SKV],
                                     ssk[64:65, 0:288], LN,
                                     bias=self.eps_row[:], scale=1.0 / D)
                rskr = self.rows.tile([1, SKV], F32, tag="srowr", bufs=1,
                                      name="rskr")
                nc.scalar.activation(rskr[0:1, :], lnk[0:1, :], EXP,
                                     bias=self.zero_row[:], scale=-0.5)
                nc.gpsimd.partition_broadcast(self.rsk_b[:, 0:288],
                                              rskr[0:1, 0:288])
                nc.gpsimd.partition_broadcast(self.rsk_b[:, 288:SKV],
                                              rskr[0:1, 288:SKV])
                kproj_copies((0, 1), pks0)
                # h0 squares on ACT (idle before first exp), ssq, rsq0,
                # q0 — fills the PE while the rsk chain resolves
                h0_sqs = self.h_squares(0, [nc.scalar])
                ssq0 = self.h_ssq_mm(0, h0_sqs)
                pks1 = kproj_mms((2, 3))
                kproj_copies((2, 3), pks1)
                self.h_rsq(0, ssq0)
                for m in range(IT):
                    self.q_proj_m(0, m)

                # transpose rsk row -> columns [kv_p, mt] via ap=1 PE
                # transposes, then scale_col = SCALE * rskT (for v)
                rskT = ps_rskt.tile([128, 8], F32, tag="rskt", name="rskT")
                for mt in range(KVT):
                    p = _kv_p(mt)
                    nc.tensor.matmul(
                        rskT[:p, mt: mt + 1],
                        rskr[0:1, mt * 128: mt * 128 + p],
                        self.ones_f[0:1, 0:1],
                        start=True, stop=True, is_transpose=True,
                    )
                self.scale_col = self.scp.tile([128, KVT], F32, tag="sc")
                nc.vector.tensor_scalar(self.scale_col[:, 0:4],
                                        rskT[:, 0:4], SCALE, None, MUL)
                nc.vector.tensor_scalar(self.scale_col[0:64, 4:5],
                                        rskT[0:64, 4:5], SCALE, None, MUL)

            with tc.tile_pool(name="wvp", bufs=1) as wvp:
                # wv fully preloaded (vproj must not be DMA-gated)
                wv_t = []
                for dp in range(DP):
                    wv_2 = wvp.tile([128, 2, INNER_L], BF16, tag=f"wv{dp}")
                    nc.sync.dma_start(out=wv_2[:],
                                      in_=self.wv[:, 2 * dp: 2 * dp + 2, :])
                    wv_t.append(wv_2)

                # h1 squares on ACT
                h1_sqs = self.h_squares(1, [nc.scalar])

                # v-proj on raw kv: dt-outer, 5 accumulators; copies apply
                # scale_col per-partition (Wv pre-scaled by sqrt(HD) so
                # scale_col*pv = rsk*kv@Wv).
                pvs = [
                    self.ps_mm.tile([128, 512], F32, tag="mm",
                                    name=f"pv{mt}")
                    for mt in range(KVT)
                ]
                for dt in range(DT):
                    for mt in range(KVT):
                        nc.tensor.matmul(
                            pvs[mt][:, :],
                            kvn(dt)[:, mt * 128: mt * 128 + 128],
                            wv_t[dt // 2][:, dt % 2, :],
                            start=(dt == 0), stop=(dt == DT - 1),
                        )
                for mt in range(KVT):
                    p = _kv_p(mt)
                    if mt % 2 == 0:
                        nc.vector.tensor_scalar(
                            self.v_sb[mt][:p, :], pvs[mt][:p, :],
                            self.scale_col[:p, mt: mt + 1], None, MUL)
                    else:
                        nc.scalar.mul(self.v_sb[mt][:p, :], pvs[mt][:p, :],
                                      self.scale_col[:p, mt: mt + 1])

                # h1 ssq + rsq1
                ssq1 = self.h_ssq_mm(1, h1_sqs)
                self.h_rsq(1, ssq1)

    # -- main chunk loop -----------------------------------------------
    def chunks(self):
        tc = self.tc
        with tc.tile_pool(name="chps", bufs=4, space="PSUM") as ch_mm, \
             tc.tile_pool(name="qps", bufs=2, space="PSUM") as ps_q, \
             tc.tile_pool(name="sups", bufs=1, space="PSUM") as ps_sums, \
             tc.tile_pool(name="attnp", bufs=1) as attnp:
            self.attnp = attnp
            self.ps_mm = ch_mm
            self.ps_q = ps_q
            self.q_bufs = 2
            self.ps_sums = ps_sums
            self.ps_row = ps_sums  # hssq rows share the sums0 bank
            self.row_tag = "sums0"
            self.sums = [None, None]
            nc = self.nc
            for qc in range(NCH):
                with nc.named_scope(f"chunk{qc}"):
                    if qc + 2 < NCH:
                        self.h_dma(qc + 2)
                    self.scores(qc, 0)
                    if qc + 1 < NCH:
                        self.q_proj_m(qc + 1, 0)
                    self.scores(qc, 1)
                    self.denoms(qc, 0)
                    if qc + 1 < NCH:
                        self.q_proj_m(qc + 1, 1)
                    self.scores(qc, 2)
                    self.denoms(qc, 1)
                    if qc + 1 < NCH:
                        self.q_proj_m(qc + 1, 2)
                    self.scores(qc, 3)
                    self.denoms(qc, 2)
                    if qc + 1 < NCH:
                        self.q_proj_m(qc + 1, 3)
                    self.denoms(qc, 3)
                    if qc + 2 < NCH:
                        sqs = self.h_squares(qc + 2, [nc.vector])
                    outT_sb = [self.attn_v_h(qc, h) for h in range(HL)]
                    if qc + 2 < NCH:
                        ssq = self.h_ssq_mm(qc + 2, sqs)
                        self.h_rsq(qc + 2, ssq)
                    self.o_proj_units(qc, outT_sb, range(16))


def _build():
    nc = bacc.Bacc()
    hT = nc.dram_tensor("ht", [NCH, DP, 128, 2, TQ], BF16,
                        kind="ExternalInput")
    kvT = nc.dram_tensor("kvt", [DP, 128, 2, KV_PAD], BF16,
                         kind="ExternalInput")
    maskb = nc.dram_tensor("maskb", [KV_PAD, 1], F32, kind="ExternalInput")
    wq = nc.dram_tensor("wq", [IT, 128, DT, 128], BF16, kind="ExternalInput")
    wk = nc.dram_tensor("wk", [IT, 128, DT, 128], BF16, kind="ExternalInput")
    wv = nc.dram_tensor("wv", [128, DT, INNER_L], BF16, kind="ExternalInput")
    wo = nc.dram_tensor("wo", [DN, 128, IT, 512], BF16, kind="ExternalInput")
    out = nc.dram_tensor("out", [T, D], BF16, kind="ExternalOutput")
    tensors = (hT, kvT, wq, wk, wv, wo, maskb, out)

    with tile.TileContext(nc, pool_alloc_mode="queue") as tc, \
         ExitStack() as ctx:
        em = _Em(nc, tc, tensors, ctx)
        em.kv_phase()
        em.chunks()

    nc.finalize()
    return nc


def _get_nc():
    global _NC
    if _NC is None:
        _NC = _build()
    return _NC


def _prep(inputs):
    hs = np.asarray(inputs["hidden_states"], np.float32)
    ws = np.asarray(inputs["workspace"], np.float32)
    corr = np.asarray(inputs["correction_tokens"], np.float32)
    cmask = np.asarray(inputs["correction_mask"])
    lnq = np.asarray(inputs["ln_q_w"], np.float32)
    lnkv = np.asarray(inputs["ln_kv_w"], np.float32)
    Wq = np.asarray(inputs["Wq"], np.float32) * lnq[:, None]
    Wk = np.asarray(inputs["Wk"], np.float32) * lnkv[:, None]
    Wv = np.asarray(inputs["Wv"], np.float32) * lnkv[:, None] * SQRT_HD
    Wo = np.asarray(inputs["Wo"], np.float32)

    import ml_dtypes
    bf16 = ml_dtypes.bfloat16

    in_maps = []
    for b in range(B):
        hT = hs[b].T.astype(bf16)                                # [D, T]
        hT = np.ascontiguousarray(
            hT.reshape(DP, 2, 128, NCH, TQ).transpose(3, 0, 2, 1, 4)
        )
        kv = np.concatenate(
            [ws[b], corr[b], np.zeros((KV_PAD - SKV, D), np.float32)],
            axis=0)                                              # [640, D]
        kvT = kv.T.astype(bf16)                                  # [D, 640]
        kvT = np.ascontiguousarray(
            kvT.reshape(DP, 2, 128, KV_PAD).transpose(0, 2, 1, 3)
        )
        mb = np.full((KV_PAD, 1), NEG, np.float32)
        mb[:N_WS] = 0.0
        mb[N_WS:SKV, 0] = np.where(cmask[b] != 0, 0.0, NEG).astype(np.float32)
        for g in range(G):
            sl = slice(g * INNER_L, (g + 1) * INNER_L)
            in_maps.append({
                "ht": hT,
                "kvt": kvT,
                "maskb": mb,
                "wq": np.ascontiguousarray(
                    Wq[:, sl].reshape(DT, 128, IT, 128).transpose(2, 1, 0, 3)
                ).astype(bf16),
                "wk": np.ascontiguousarray(
                    Wk[:, sl].reshape(DT, 128, IT, 128).transpose(2, 1, 0, 3)
                ).astype(bf16),
                "wv": np.ascontiguousarray(
                    Wv[:, sl].reshape(DT, 128, INNER_L).transpose(1, 0, 2)
                ).astype(bf16),
                "wo": np.ascontiguousarray(
                    Wo[sl, :].reshape(IT, 128, DN, 512).transpose(2, 1, 0, 3)
                ).astype(bf16),
            })
    return in_maps


def kernel(**inputs):
    global LAST_RESULTS
    nc = _get_nc()
    in_maps = _prep(inputs)
    trace = os.environ.get("KERNEL_TRACE", "0") == "1"
    res = run_bass_kernel_spmd(
        nc, in_maps, core_ids=list(range(B * G)),
        trace=trace, trace_cores=[0] if trace else None,
    )
    LAST_RESULTS = res
    parts = [r["out"] for r in res.results]
    out = np.empty((B, T, D), np.float32)
    for b in range(B):
        out[b] = np.sum(
            np.stack([p.astype(np.float32) for p in parts[b * G: (b + 1) * G]]),
            axis=0, dtype=np.float32)
    return out


# revision 75
# speedup vs baseline: 1.0053x; 1.0053x over previous
"""Cross-attention adapter kernel for Trainium2 (8 NeuronCores).

Sharding: data-parallel over batch (B=2) x tensor-parallel over heads
(4 groups of 4 heads). Core c handles batch c//4, head group c%4. Each
core computes rmsnorm + q/k/v projections for its head slice, attention,
and a partial o-projection; the host sums the 4 partials per batch.

v3 scheduling (vs v2 baseline at ~334us; measured ~262us):
- k/v projections run on RAW kv (no in-place normalize): rsk is folded
  into the kT psum->sbuf copies (tensor_mul with an rsk broadcast) and
  into the v copies (per-partition tensor_scalar with a transposed rsk
  column built from 5 ap=1 PE transposes; Wv is host-prescaled by
  sqrt(HD) so one column serves both). Removes the kv-normalize DVE
  serialization and its PE stall, and lets k-proj start as soon as
  wk/kv DMAs land.
- rsk itself is exp(-0.5*ln(ssq/D+eps)) on ACT (Ln/Exp share the Exp
  table -> no ACT table swaps anywhere in the kernel).
- Per-chunk emission interleaves q-proj m-tiles between per-head
  scores groups so the PE never waits on the ACT exp chain; ssq
  matmuls are emitted late (their squares long done on vector).
- kv is zero-padded to 640 tokens host-side so scores / attn_v /
  v-proj tail tiles run full-width (128 partitions) instead of the
  slow 64-row path; the mask bias covers the pad.
- h squares for prefetched chunks run on vector; h0/h1 squares in the
  kv phase run on the then-idle ACT engine. hp pool holds 3 chunk
  buffers so the h(qc+2) DMA is never blocked by q-proj(qc+1).
- o-proj partials are written as bf16 (host sums in fp32): halves the
  output DMA; the last chunk's final out-DMAs split across two queues.
- Denominator rows for 2 heads share one PSUM bank (partitions 0/64);
  the hssq row shares the sums0 bank (temporally disjoint), freeing a
  bank for a 4-deep scores/attn_v/o-proj psum rotation.

Known plateau: PE busy ~229us (bf16 matmul-column floor ~206us) +
~12us startup DMA + ~13us residual gaps + ~7us drain tail. fp8 was
evaluated and rejected: e4m3 quantization noise exceeds the 2e-2
correctness gate (measured 5.8-11% in simulation).
"""

import sys

if "/opt/trn_rl_repo" not in sys.path:
    sys.path.insert(0, "/opt/trn_rl_repo")

import os
import types
from contextlib import ExitStack

import numpy as np


def _install_ntff_shim():
    """Make `antenv.axon_hooks` importable when the image lacks it, so
    run_bass_kernel_spmd(trace=True) can capture NTFF profiles under
    axon. No-op when the real module exists or axon boot is absent."""
    try:
        import antenv.axon_hooks  # noqa: F401
        return
    except ImportError:
        pass
    try:
        import antenv
        from trn_agent_boot.trn_boot import _ntff_profile_via_ctypes
        hook = _ntff_profile_via_ctypes("/opt/axon/libaxon_pjrt.so")
        mod = types.ModuleType("antenv.axon_hooks")
        mod._hook = hook
        mod.get_axon_ntff_profile_hook = lambda: mod._hook
        mod.set_axon_ntff_profile_hook = lambda h: setattr(mod, "_hook", h)
        import sys as _sys
        _sys.modules["antenv.axon_hooks"] = mod
        antenv.axon_hooks = mod
    except Exception:
        pass


_install_ntff_shim()

import concourse.mybir as mybir
import concourse.tile as tile
from concourse import bacc
from concourse.bass_utils import run_bass_kernel_spmd

# Problem shapes (hardcoded per contest contract).
B = 2
T = 2048
D = 2048
H = 16
HD = 128
N_WS = 64
T_CORR = 512
SKV = N_WS + T_CORR  # 576
EPS = 1e-6

# Sharding config.
G = 4                  # tensor-parallel head groups
HL = H // G            # 4 local heads per core
INNER_L = HL * HD      # 512
IT = INNER_L // 128    # 4  m-tiles of local inner dim
TQ = 512               # query-token chunk
NCH = T // TQ          # 4
DT = D // 128          # 16 contraction tiles over D
DP = DT // 2           # 8 dt-pair tiles
KVT = (SKV + 127) // 128       # 5 kv partition tiles
KV_PAD = KVT * 128             # 640
DN = D // 512          # 4  output N-tiles
SCALE = 1.0 / float(np.sqrt(HD))
SQRT_HD = float(np.sqrt(HD))
NEG = -1.0e9

F32 = mybir.dt.float32
BF16 = mybir.dt.bfloat16
EXP = mybir.ActivationFunctionType.Exp
SQUARE = mybir.ActivationFunctionType.Square
MUL = mybir.AluOpType.mult
ADD = mybir.AluOpType.add

_NC = None
LAST_RESULTS = None


def _kv_p(mt):
    return min(128, SKV - mt * 128)


class _Em:
    def __init__(self, nc, tc, tensors, ctx):
        self.nc = nc
        self.tc = tc
        (self.hT, self.kvT, self.wq, self.wk, self.wv, self.wo,
         self.maskb, self.out) = tensors

        p = lambda name, bufs: ctx.enter_context(
            tc.tile_pool(name=name, bufs=bufs))
        self.const = p("const", 1)
        self.rows = p("rows", 4)
        self.bcast = p("bc", 2)       # per-tag bufs
        self.ktp = p("kt", 1)
        self.vp = p("v", 1)
        self.wqp = p("wq", 1)
        self.wop = p("wo", 1)
        self.hp = p("hp", 3)
        self.hsqp = p("hsq", 1)       # per-dt tags, h squares all phases
        self.qp = p("qp", 1)
        self.otp = p("otp", 1)
        self.fop = p("fop", 4)
        self.scp = p("scp", 1)
        # transient pools (kv phase / chunk scope), set later
        self.sqp = None
        self.attnp = None

        self.h_ts = {}
        self.rsq = {}
        self.qT = {}
        self.kT_sb = None
        self.v_sb = None
        self.scale_col = None
        self.attn = {}
        self.rsum_b = {}
        self.fo_eng = 0

        # constants
        nc_ = self.nc
        self.ones_f = self.const.tile([128, 1], F32)
        nc_.vector.memset(self.ones_f[:], 1.0)
        self.ones_t = self.const.tile([128, 1], BF16)
        nc_.vector.tensor_copy(self.ones_t[:], self.ones_f[:])
        self.eps_row = self.const.tile([1, 1], F32)
        nc_.vector.memset(self.eps_row[:], EPS)
        self.zero_row = self.const.tile([1, 1], F32)
        nc_.vector.memset(self.zero_row[:], 0.0)
        self.maskb_sb = self.const.tile([128, KVT], F32)
        nc_.scalar.dma_start(
            out=self.maskb_sb[:],
            in_=self.maskb.rearrange("(mt p) one -> p (mt one)", p=128),
        )


    # -- h pipeline ----------------------------------------------------
    def h_dma(self, qc):
        nc = self.nc
        ht = self.hp.tile([128, DP, 2, TQ], BF16, tag="h", name=f"h{qc}")
        for dp in range(DP):
            nc.sync.dma_start(out=ht[:, dp, :, :], in_=self.hT[qc, dp])
        self.h_ts[qc] = ht
        return ht

    def h_squares(self, qc, engines):
        """Square h dt-tiles; engines is a list cycled over dt."""
        nc = self.nc
        ht = self.h_ts[qc]
        sqs = []
        for dt in range(DT):
            sq = self.hsqp.tile([128, TQ], BF16, tag=f"hsq{dt}",
                                name=f"hsq{qc}_{dt}")
            eng = engines[dt % len(engines)]
            if eng is nc.scalar:
                nc.scalar.activation(sq[:], ht[:, dt // 2, dt % 2, :],
                                     SQUARE)
            else:
                eng.tensor_mul(sq[:], ht[:, dt // 2, dt % 2, :],
                               ht[:, dt // 2, dt % 2, :])
            sqs.append(sq)
        return sqs

    def h_ssq_mm(self, qc, sqs):
        nc = self.nc
        ssq = self.ps_row.tile([128, TQ], F32, tag=self.row_tag,
                               name=f"ssq{qc}")
        for dt in range(DT):
            nc.tensor.matmul(
                ssq[0:1, :], self.ones_t[:], sqs[dt][:],
                start=(dt == 0), stop=(dt == DT - 1),
            )
        return ssq

    def h_rsq(self, qc, ssq):
        """rsq = 1/sqrt(ssq/D + eps) via two Newton steps from y0=1."""
        nc = self.nc
        y1 = self.rows.tile([1, SKV], F32, tag="srow", name=f"y1h{qc}")
        m = self.rows.tile([1, SKV], F32, tag="srow", name=f"mh{qc}")
        t = self.rows.tile([1, SKV], F32, tag="srow", name=f"th{qc}")
        nc.vector.tensor_scalar(y1[0:1, 0:TQ], ssq[0:1, :],
                                -0.5 / D, 1.5 - 0.5 * EPS, MUL, ADD)
        nc.vector.tensor_scalar(m[0:1, 0:TQ], ssq[0:1, :],
                                1.0 / D, EPS, MUL, ADD)
        nc.vector.tensor_mul(t[0:1, 0:TQ], y1[0:1, 0:TQ], y1[0:1, 0:TQ])
        nc.vector.tensor_mul(t[0:1, 0:TQ], t[0:1, 0:TQ], m[0:1, 0:TQ])
        nc.vector.tensor_scalar(t[0:1, 0:TQ], t[0:1, 0:TQ],
                                -0.5, 1.5, MUL, ADD)
        nc.vector.tensor_mul(y1[0:1, 0:TQ], y1[0:1, 0:TQ], t[0:1, 0:TQ])
        rsq_b = self.bcast.tile([128, SKV], F32, tag="rsqb",
                                name=f"rsqb{qc}")
        nc.gpsimd.partition_broadcast(rsq_b[:, 0:TQ], y1[0:1, 0:TQ])
        self.rsq[qc] = rsq_b

    # -- projections ---------------------------------------------------
    def q_proj_m(self, qc, m):
        nc = self.nc
        ht = self.h_ts[qc]
        pq = self.ps_q.tile([128, 512], F32, tag="qmm", bufs=self.q_bufs)
        for dt in range(DT):
            nc.tensor.matmul(
                pq[:, 0:TQ],
                self.wq_sb[:, m, dt, :],
                ht[:, dt // 2, dt % 2, :],
                start=(dt == 0), stop=(dt == DT - 1),
            )
        qt = self.qp.tile([128, TQ], BF16, tag=f"q{m}", name=f"q{qc}_{m}")
        nc.vector.tensor_mul(qt[:], pq[:, 0:TQ], self.rsq[qc][:, 0:TQ])
        self.qT.setdefault(qc, [None] * IT)[m] = qt
        if m == IT - 1:
            self.rsq.pop(qc)

    # -- attention -----------------------------------------------------
    def scores(self, qc, hl):
        """scores + exp for one head; exp folds SCALE*rsk per kv row."""
        nc = self.nc
        qt = self.qT[qc][hl]
        attn_sb = self.attnp.tile([128, KVT, TQ], BF16, tag=f"attn{hl}",
                                  name=f"attn{qc}_{hl}")
        for mt in range(KVT):
            ps = self.ps_mm.tile([128, 512], F32, tag="mm")
            nc.tensor.matmul(
                ps[:, 0:TQ],
                self.kT_sb[hl][:, mt * 128: mt * 128 + 128],
                qt[:],
                start=True, stop=True,
            )
            nc.scalar.activation(
                attn_sb[:, mt, :], ps[:, 0:TQ], EXP,
                bias=self.maskb_sb[:, mt: mt + 1], scale=SCALE,
            )
        self.attn[(qc, hl)] = attn_sb

    def denoms(self, qc, hl):
        """ones-matmul row sums for one head + reciprocal broadcast.
        Two heads share one PSUM bank (rows 0 / 64)."""
        nc = self.nc
        attn_sb = self.attn[(qc, hl)]
        grp = hl // 2
        if hl % 2 == 0:
            self.sums[grp] = self.ps_sums.tile(
                [128, TQ], F32, tag=f"sums{grp}", bufs=1,
                name=f"sums{qc}_{grp}")
        srow = self.sums[grp][(hl % 2) * 64: (hl % 2) * 64 + 1, :]
        for mt in range(KVT):
            nc.tensor.matmul(
                srow, self.ones_t[:], attn_sb[:, mt, :],
                start=(mt == 0), stop=(mt == KVT - 1),
            )
        rsb = self.rows.tile([1, SKV], F32, tag="srow",
                             name=f"rsb{qc}_{hl}")
        rs = self.rows.tile([1, SKV], F32, tag="srow",
                            name=f"rs{qc}_{hl}")
        nc.vector.tensor_copy(rsb[0:1, 0:TQ], srow)
        nc.vector.reciprocal_approx_fast(rs[0:1, 0:TQ], rsb[0:1, 0:TQ])
        rb = self.bcast.tile([128, SKV], F32, tag=f"rsum{hl % 2}",
                             name=f"rsb{qc}_{hl}")
        nc.gpsimd.partition_broadcast(rb[:, 0:TQ], rs[0:1, 0:TQ])
        self.rsum_b[(qc, hl)] = rb

    def attn_v_h(self, qc, hl):
        nc = self.nc
        attn_sb = self.attn.pop((qc, hl))
        po = self.ps_mm.tile([128, 512], F32, tag="mm")
        for mt in range(KVT):
            nc.tensor.matmul(
                po[:, 0:TQ],
                self.v_sb[mt][:, hl * 128: (hl + 1) * 128],
                attn_sb[:, mt, :],
                start=(mt == 0), stop=(mt == KVT - 1),
            )
        ot = self.otp.tile([128, TQ], BF16, tag=f"ot{hl}",
                           name=f"ot{qc}_{hl}")
        rb = self.rsum_b.pop((qc, hl))
        nc.vector.tensor_mul(ot[:], po[:, 0:TQ], rb[:, 0:TQ])
        return ot

    # -- o-projection --------------------------------------------------
    def o_proj_units(self, qc, outT_sb, units):
        """Emit a subset of the 16 (tt, n) o-projection units."""
        nc = self.nc
        for u in units:
            tt, n = u // DN, u % DN
            pf = self.ps_mm.tile([128, 512], F32, tag="mm")
            for m in range(IT):
                nc.tensor.matmul(
                    pf[:],
                    outT_sb[m][:, tt * 128: (tt + 1) * 128],
                    self.wo_sb[:, n, m, :],
                    start=(m == 0), stop=(m == IT - 1),
                )
            fo = self.fop.tile([128, 512], BF16, tag="fo")
            e = self.fo_eng % 2
            self.fo_eng += 1
            if e == 0:
                nc.scalar.copy(fo[:], pf[:])
                deng = nc.scalar
            else:
                nc.vector.tensor_copy(fo[:], pf[:])
                deng = nc.gpsimd
            t0 = qc * TQ + tt * 128
            if qc == NCH - 1 and u >= 8:
                # tail: split across two queues to halve drain time
                deng.dma_start(
                    out=self.out[t0: t0 + 64, n * 512: (n + 1) * 512],
                    in_=fo[0:64, :],
                )
                deng2 = nc.gpsimd if deng is nc.scalar else nc.scalar
                deng2.dma_start(
                    out=self.out[t0 + 64: t0 + 128,
                                 n * 512: (n + 1) * 512],
                    in_=fo[64:128, :],
                )
            else:
                deng.dma_start(
                    out=self.out[t0: t0 + 128, n * 512: (n + 1) * 512],
                    in_=fo[:],
                )

    # -- kv phase ------------------------------------------------------
    def kv_phase(self):
        nc, tc = self.nc, self.tc
        with tc.tile_pool(name="kvps", bufs=5, space="PSUM") as kv_mm, \
             tc.tile_pool(name="kvrow", bufs=1, space="PSUM") as ps_row, \
             tc.tile_pool(name="rskt", bufs=1, space="PSUM") as ps_rskt, \
             nc.named_scope("kvphase"):
            self.ps_mm = kv_mm
            self.ps_q = kv_mm
            self.q_bufs = 1
            self.ps_row = ps_row
            self.row_tag = "row"
            self._kv_phase_body(ps_rskt)

    def _kv_phase_body(self, ps_rskt):
        nc, tc = self.nc, self.tc
        self.kT_sb = [
            self.ktp.tile([128, KV_PAD], BF16, tag=f"kt{m}", name=f"kt{m}")
            for m in range(IT)
        ]
        self.v_sb = [
            self.vp.tile([128, INNER_L], BF16, tag=f"v{mt}", name=f"v{mt}")
            for mt in range(KVT)
        ]
        # zero the kv padding (tokens 576..639) so the padded scores /
        # attn_v tiles contribute exactly zero
        for m in range(IT):
            nc.gpsimd.memset(self.kT_sb[m][:, SKV:KV_PAD], 0.0)
        nc.gpsimd.memset(self.v_sb[KVT - 1][64:128, :], 0.0)
        wk_r = self.wk.rearrange("m p dt c -> p m dt c")
        wq_r = self.wq.rearrange("m p dt c -> p m dt c")
        wo_r = self.wo.rearrange("n p m c -> p n m c")
        with tc.tile_pool(name="kvp", bufs=1) as kvp, \
             tc.tile_pool(name="sqkv", bufs=1) as sqkv:
            kv_t = kvp.tile([128, DP, 2, KV_PAD], BF16, tag="kv", name="kv")

            def kvn(dt):
                return kv_t[:, dt // 2, dt % 2, :]

            with tc.tile_pool(name="wkp", bufs=1) as wkp:
                # DMA order: wk(m)/kv(dp) interleaved so k-proj's first
                # m-pass is fed earliest
                wk_sb = wkp.tile([128, IT, DT, 128], BF16, tag="wk")
                for m in range(IT):
                    for hh in range(2):
                        nc.sync.dma_start(
                            out=wk_sb[:, m, hh * 8:(hh + 1) * 8, :],
                            in_=wk_r[:, m, hh * 8:(hh + 1) * 8, :])
                    for dp in (2 * m, 2 * m + 1):
                        nc.sync.dma_start(out=kv_t[:, dp, :, :],
                                          in_=self.kvT[dp])
                self.h_dma(0)
                self.wq_sb = self.wqp.tile([128, IT, DT, 128], BF16,
                                           tag="wq")
                for m in range(IT):
                    nc.sync.dma_start(out=self.wq_sb[:, m, :, :],
                                      in_=wq_r[:, m, :, :])
                self.h_dma(1)
                self.wo_sb = self.wop.tile([128, DN, IT, 512], BF16,
                                           tag="wo")
                for n in range(DN):
                    nc.sync.dma_start(out=self.wo_sb[:, n, :, :],
                                      in_=wo_r[:, n, :, :])

                self.rsk_b = self.bcast.tile([128, SKV], F32, tag="rskb",
                                             bufs=1, name="rskb")

                def kproj_mms(ms):
                    pks = {}
                    for m in ms:
                        for i in range(2):
                            pks[(m, i)] = self.ps_mm.tile(
                                [128, 512], F32, tag="mm",
                                name=f"pk{m}_{i}")
                    for dt in range(DT):
                        for m in ms:
                            for i, (s0, s1) in enumerate(
                                    ((0, 288), (288, SKV))):
                                nc.tensor.matmul(
                                    pks[(m, i)][:, 0: s1 - s0],
                                    wk_sb[:, m, dt, :],
                                    kvn(dt)[:, s0:s1],
                                    start=(dt == 0), stop=(dt == DT - 1),
                                )
                    return pks

                def kproj_copies(ms, pks):
                    # rsk folded at copy time via rsk_b broadcast
                    for m in ms:
                        for i, (s0, s1) in enumerate(((0, 288), (288, SKV))):
                            nc.vector.tensor_mul(
                                self.kT_sb[m][:, s0:s1],
                                pks[(m, i)][:, 0: s1 - s0],
                                self.rsk_b[:, s0:s1])

                pks0 = kproj_mms((0, 1))

                # kv squares on vector/gpsimd; ssq matmuls accumulate two
                # 288-halves at psum partitions 0/64
                ssk = self.ps_row.tile([128, 512], F32, tag="row",
                                       name="ssk")
                for dt in range(DT):
                    sq = sqkv.tile([128, SKV], BF16, tag=f"kvsq{dt}",
                                   name=f"kvsq{dt}")
                    eng = nc.vector if dt % 2 == 0 else nc.gpsimd
                    eng.tensor_mul(sq[:], kvn(dt)[:, 0:SKV],
                                   kvn(dt)[:, 0:SKV])
                    nc.tensor.matmul(
                        ssk[0:1, 0:288], self.ones_t[:], sq[:, 0:288],
                        start=(dt == 0), stop=(dt == DT - 1),
                    )
                    nc.tensor.matmul(
                        ssk[64:65, 0:288], self.ones_t[:], sq[:, 288:SKV],
                        start=(dt == 0), stop=(dt == DT - 1),
                    )

                # rsk = exp(-0.5*ln(ssq/D + eps)) on ACT (Ln/Exp share
                # the Exp table -> no table swap; much shorter chain
                # than DVE newton rows)
                lnk = self.rows.tile([1, SKV], F32, tag="srow",
                                     name="lnk")
                LN = mybir.ActivationFunctionType.Ln
                nc.scalar.activation(lnk[0:1, 0:288],
                                     ssk[0:1, 0:288], LN,
                                     bias=self.eps_row[:], scale=1.0 / D)
                nc.scalar.activation(lnk[0:1, 288:SKV],
                                     ssk[64:65, 0:288], LN,
                                     bias=self.eps_row[:], scale=1.0 / D)
                rskr = self.rows.tile([1, SKV], F32, tag="srowr", bufs=1,
                                      name="rskr")
                nc.scalar.activation(rskr[0:1, :], lnk[0:1, :], EXP,
                                     bias=self.zero_row[:], scale=-0.5)
                nc.gpsimd.partition_broadcast(self.rsk_b[:, 0:288],
                                              rskr[0:1, 0:288])
                nc.gpsimd.partition_broadcast(self.rsk_b[:, 288:SKV],
                                              rskr[0:1, 288:SKV])
                kproj_copies((0, 1), pks0)
                # h0 squares on ACT (idle before first exp), ssq, rsq0,
                # q0 — fills the PE while the rsk chain resolves
                h0_sqs = self.h_squares(0, [nc.scalar])
                ssq0 = self.h_ssq_mm(0, h0_sqs)
                pks1 = kproj_mms((2, 3))
                kproj_copies((2, 3), pks1)
                self.h_rsq(0, ssq0)
                for m in range(IT):
                    self.q_proj_m(0, m)

                # transpose rsk row -> columns [kv_p, mt] via ap=1 PE
                # transposes, then scale_col = SCALE * rskT (for v)
                rskT = ps_rskt.tile([128, 8], F32, tag="rskt", name="rskT")
                for mt in range(KVT):
                    p = _kv_p(mt)
                    nc.tensor.matmul(
                        rskT[:p, mt: mt + 1],
                        rskr[0:1, mt * 128: mt * 128 + p],
                        self.ones_f[0:1, 0:1],
                        start=True, stop=True, is_transpose=True,
                    )
                self.scale_col = self.scp.tile([128, KVT], F32, tag="sc")
                nc.vector.tensor_scalar(self.scale_col[:, 0:4],
                                        rskT[:, 0:4], SCALE, None, MUL)
                nc.vector.tensor_scalar(self.scale_col[0:64, 4:5],
                                        rskT[0:64, 4:5], SCALE, None, MUL)

            with tc.tile_pool(name="wvp", bufs=1) as wvp:
                # wv fully preloaded (vproj must not be DMA-gated)
                wv_t = []
                for dp in range(DP):
                    wv_2 = wvp.tile([128, 2, INNER_L], BF16, tag=f"wv{dp}")
                    nc.sync.dma_start(out=wv_2[:],
                                      in_=self.wv[:, 2 * dp: 2 * dp + 2, :])
                    wv_t.append(wv_2)

                # h1 squares on ACT
                h1_sqs = self.h_squares(1, [nc.scalar])

                # v-proj on raw kv: dt-outer, 5 accumulators; copies apply
                # scale_col per-partition (Wv pre-scaled by sqrt(HD) so
                # scale_col*pv = rsk*kv@Wv).
                pvs = [
                    self.ps_mm.tile([128, 512], F32, tag="mm",
                                    name=f"pv{mt}")
                    for mt in range(KVT)
                ]
                for dt in range(DT):
                    for mt in range(KVT):
                        nc.tensor.matmul(
                            pvs[mt][:, :],
                            kvn(dt)[:, mt * 128: mt * 128 + 128],
                            wv_t[dt // 2][:, dt % 2, :],
                            start=(dt == 0), stop=(dt == DT - 1),
                        )
                for mt in range(KVT):
                    p = _kv_p(mt)
                    if mt % 2 == 0:
                        nc.vector.tensor_scalar(
                            self.v_sb[mt][:p, :], pvs[mt][:p, :],
                            self.scale_col[:p, mt: mt + 1], None, MUL)
                    else:
                        nc.scalar.mul(self.v_sb[mt][:p, :], pvs[mt][:p, :],
                                      self.scale_col[:p, mt: mt + 1])

                # h1 ssq + rsq1
                ssq1 = self.h_ssq_mm(1, h1_sqs)
                self.h_rsq(1, ssq1)

    # -- main chunk loop -----------------------------------------------
    def chunks(self):
        tc = self.tc
        with tc.tile_pool(name="chps", bufs=3, space="PSUM") as ch_mm, \
             tc.tile_pool(name="qps", bufs=2, space="PSUM") as ps_q, \
             tc.tile_pool(name="sups", bufs=1, space="PSUM") as ps_sums, \
             tc.tile_pool(name="chrow", bufs=1, space="PSUM") as ps_row, \
             tc.tile_pool(name="attnp", bufs=1) as attnp:
            self.attnp = attnp
            self.ps_mm = ch_mm
            self.ps_q = ps_q
            self.q_bufs = 2
            self.ps_sums = ps_sums
            self.ps_row = ps_row
            self.row_tag = "row"
            self.sums = [None, None]
            nc = self.nc
            for qc in range(NCH):
                with nc.named_scope(f"chunk{qc}"):
                    if qc + 2 < NCH:
                        self.h_dma(qc + 2)
                    self.scores(qc, 0)
                    if qc + 1 < NCH:
                        self.q_proj_m(qc + 1, 0)
                    self.scores(qc, 1)
                    self.denoms(qc, 0)
                    if qc + 1 < NCH:
                        self.q_proj_m(qc + 1, 1)
                    self.scores(qc, 2)
                    self.denoms(qc, 1)
                    if qc + 1 < NCH:
                        self.q_proj_m(qc + 1, 2)
                    self.scores(qc, 3)
                    self.denoms(qc, 2)
                    if qc + 1 < NCH:
                        self.q_proj_m(qc + 1, 3)
                    self.denoms(qc, 3)
                    if qc + 2 < NCH:
                        sqs = self.h_squares(qc + 2, [nc.vector])
                    outT_sb = [self.attn_v_h(qc, h) for h in range(HL)]
                    if qc + 2 < NCH:
                        ssq = self.h_ssq_mm(qc + 2, sqs)
                        self.h_rsq(qc + 2, ssq)
                    self.o_proj_units(qc, outT_sb, range(16))


def _build():
    nc = bacc.Bacc()
    hT = nc.dram_tensor("ht", [NCH, DP, 128, 2, TQ], BF16,
                        kind="ExternalInput")
    kvT = nc.dram_tensor("kvt", [DP, 128, 2, KV_PAD], BF16,
                         kind="ExternalInput")
    maskb = nc.dram_tensor("maskb", [KV_PAD, 1], F32, kind="ExternalInput")
    wq = nc.dram_tensor("wq", [IT, 128, DT, 128], BF16, kind="ExternalInput")
    wk = nc.dram_tensor("wk", [IT, 128, DT, 128], BF16, kind="ExternalInput")
    wv = nc.dram_tensor("wv", [128, DT, INNER_L], BF16, kind="ExternalInput")
    wo = nc.dram_tensor("wo", [DN, 128, IT, 512], BF16, kind="ExternalInput")
    out = nc.dram_tensor("out", [T, D], BF16, kind="ExternalOutput")
    tensors = (hT, kvT, wq, wk, wv, wo, maskb, out)

    with tile.TileContext(nc, pool_alloc_mode="queue") as tc, \
         ExitStack() as ctx:
        em = _Em(nc, tc, tensors, ctx)
        em.kv_phase()
        em.chunks()

    nc.finalize()
    return nc


def _get_nc():
    global _NC
    if _NC is None:
        _NC = _build()
    return _NC


def _prep(inputs):
    hs = np.asarray(inputs["hidden_states"], np.float32)
    ws = np.asarray(inputs["workspace"], np.float32)
    corr = np.asarray(inputs["correction_tokens"], np.float32)
    cmask = np.asarray(inputs["correction_mask"])
    lnq = np.asarray(inputs["ln_q_w"], np.float32)
    lnkv = np.asarray(inputs["ln_kv_w"], np.float32)
    Wq = np.asarray(inputs["Wq"], np.float32) * lnq[:, None]
    Wk = np.asarray(inputs["Wk"], np.float32) * lnkv[:, None]
    Wv = np.asarray(inputs["Wv"], np.float32) * lnkv[:, None] * SQRT_HD
    Wo = np.asarray(inputs["Wo"], np.float32)

    import ml_dtypes
    bf16 = ml_dtypes.bfloat16

    in_maps = []
    for b in range(B):
        hT = hs[b].T.astype(bf16)                                # [D, T]
        hT = np.ascontiguousarray(
            hT.reshape(DP, 2, 128, NCH, TQ).transpose(3, 0, 2, 1, 4)
        )
        kv = np.concatenate(
            [ws[b], corr[b], np.zeros((KV_PAD - SKV, D), np.float32)],
            axis=0)                                              # [640, D]
        kvT = kv.T.astype(bf16)                                  # [D, 640]
        kvT = np.ascontiguousarray(
            kvT.reshape(DP, 2, 128, KV_PAD).transpose(0, 2, 1, 3)
        )
        mb = np.full((KV_PAD, 1), NEG, np.float32)
        mb[:N_WS] = 0.0
        mb[N_WS:SKV, 0] = np.where(cmask[b] != 0, 0.0, NEG).astype(np.float32)
        for g in range(G):
            sl = slice(g * INNER_L, (g + 1) * INNER_L)
            in_maps.append({
                "ht": hT,
                "kvt": kvT,
                "maskb": mb,
                "wq": np.ascontiguousarray(
                    Wq[:, sl].reshape(DT, 128, IT, 128).transpose(2, 1, 0, 3)
                ).astype(bf16),
                "wk": np.ascontiguousarray(
                    Wk[:, sl].reshape(DT, 128, IT, 128).transpose(2, 1, 0, 3)
                ).astype(bf16),
                "wv": np.ascontiguousarray(
                    Wv[:, sl].reshape(DT, 128, INNER_L).transpose(1, 0, 2)
                ).astype(bf16),
                "wo": np.ascontiguousarray(
                    Wo[sl, :].reshape(IT, 128, DN, 512).transpose(2, 1, 0, 3)
                ).astype(bf16),
            })
    return in_maps


def kernel(**inputs):
    global LAST_RESULTS
    nc = _get_nc()
    in_maps = _prep(inputs)
    trace = os.environ.get("KERNEL_TRACE", "0") == "1"
    res = run_bass_kernel_spmd(
        nc, in_maps, core_ids=list(range(B * G)),
        trace=trace, trace_cores=[0] if trace else None,
    )
    LAST_RESULTS = res
    parts = [r["out"] for r in res.results]
    out = np.empty((B, T, D), np.float32)
    for b in range(B):
        out[b] = np.sum(
            np.stack([p.astype(np.float32) for p in parts[b * G: (b + 1) * G]]),
            axis=0, dtype=np.float32)
    return out
